# revision 10
# baseline (speedup 1.0000x reference)
"""Trainium2 Bass kernel for nn_CUFLayer_83640193122985.

CUF layer: per-pixel hypernet MLP (118->32->32->32->32->2304) generates 3x3
per-channel kernels at each of 128x128 target pixels; applied to the 2x
nearest-upsampled main_input [4,64,64,256]; then 1x1 projection [256->128].

Key algebraic optimization (parity decomposition): the upsample is exactly 2x
nearest-neighbor, so each output pixel's 3x3 window covers only 2x2 DISTINCT
source pixels; which taps collapse onto which source pixel depends only on the
output pixel's (row, col) parity. W_out/b_out columns are pre-combined on the
host per parity class, turning 9 multiply-taps into 4 and letting the whole
apply stage run at source resolution.

Sharding: 8-way data parallel over output rows (16 rows/core, all batches),
hypernet recomputed per-core for its slab; no collectives. The DCT feature
matrix is input-independent and precomputed on host. Matmuls run in float32r
(near-fp32 precision at full PE rate); the per-pixel multiply runs in bf16 on
the vector engine (2x packed mode, batch-broadcast); tap and channel
accumulation ride the PE's PSUM accumulation fused with the 1x1 projection.
Inputs are packed into few DRAM tensors in need-order (HWDGE dispatch is
~0.65us/DMA, transfers serialize at ~360GB/s); per-class outputs accumulate in
two 2-bank PSUM batch-pair tiles whose copy+DMA overlap each other's matmuls.

Self-contained: hardcodes all shapes; no sibling imports.
"""

import numpy as np
import ml_dtypes

import concourse.bass as bass
import concourse.mybir as mybir
import concourse.tile as tile
from concourse import bacc
from concourse import bass_utils

BF16 = ml_dtypes.bfloat16
F32R = mybir.dt.float32r

K = 3
DCT_BASIS = 25
B, H_IN, W_IN, C = 4, 64, 64, 256
H_T, W_T, F_OUT = 128, 128, 128
N_CORES = 8
RPC = H_T // N_CORES  # 16 output rows per core
D_IN = 118
NPIX = RPC * W_T  # 2048 pixels per core
MROWS = RPC // 2 + 2  # 10 source rows incl halo
MCOLS = W_IN + 2  # 66 source cols incl halo
QR = RPC // 2  # 8 source-row positions per core
QC = W_IN  # 64 source-col positions

# vertical tap-collapse table: V[pi][ai] = (alpha, [di...]); same for cols
_V = {0: [(-1, [0]), (0, [1, 2])], 1: [(0, [0, 1]), (1, [2])]}
_CLASSES = [(0, 0), (0, 1), (1, 0), (1, 1)]

_CACHE: dict = {}


# ----------------------------------------------------------------- host side
def _build_features():
    """feat [H_T, W_T, 118] fp32 — input-independent constant."""
    f = np.linspace(1.0, 2.0, DCT_BASIS).astype(np.float32)
    gh = np.linspace(0.0, 1.0, H_T).astype(np.float32)
    row_enc = np.cos(np.pi * (2.0 * gh[:, None] + 1.0) * f[None, :]).astype(np.float32)
    delta = np.concatenate(
        [
            np.broadcast_to(row_enc[:, None, :], (H_T, W_T, DCT_BASIS)),
            np.broadcast_to(row_enc[None, :, :], (H_T, W_T, DCT_BASIS)),
        ],
        axis=-1,
    )
    scale = np.array([H_T / H_IN, W_T / W_IN], np.float32)
    scale_enc = np.cos(np.pi * (2.0 * scale[:, None] + 1.0) * f[None, :]).reshape(-1)
    offs = np.arange(K, dtype=np.float32) - 1.0
    ki, kj = np.meshgrid(offs, offs, indexing="ij")
    kidx = np.stack([ki, kj], -1).reshape(K * K, 2)
    f9 = np.linspace(1.0, 1.0, 9).astype(np.float32)
    kenc = np.cos(np.pi * (2.0 * kidx[..., None] + 1.0) * f9).reshape(K * K, 18).mean(0)
    feat = np.concatenate(
        [
            delta,
            np.broadcast_to(scale_enc, (H_T, W_T, 50)),
            np.broadcast_to(kenc.astype(np.float32), (H_T, W_T, 18)),
        ],
        axis=-1,
    ).astype(np.float32)
    return feat  # [128,128,118]


def _chunk_meta():
    """Per combined-kernel chunk m = class*8 + A*2 + cc: (class, pi, pj,
    alpha, beta, cc, taps). A = ai*2 + bi."""
    meta = []
    for ci, (pi, pj) in enumerate(_CLASSES):
        for ai in range(2):
            for bi in range(2):
                alpha, dis = _V[pi][ai]
                beta, djs = _V[pj][bi]
                taps = [di * 3 + dj for di in dis for dj in djs]
                for cc in range(2):
                    meta.append((ci, pi, pj, alpha, beta, cc, taps))
    return meta


def _host_prep(inputs):
    """Build per-core input maps (few, large tensors to minimize DMA count)."""
    main_input = np.asarray(inputs["main_input"], np.float32)
    feat = _CACHE.get("feat")
    if feat is None:
        feat = _CACHE["feat"] = _build_features()

    # source image, zero-padded by 1: [B, 66, 66, C] then channel-major bf16
    mp = np.pad(main_input, ((0, 0), (1, 1), (1, 1), (0, 0)))

    Wout = np.asarray(inputs["W_out"], np.float32)  # [32, 2304] cols t*256+c
    bout = np.asarray(inputs["b_out"], np.float32)
    wcomb = np.empty((32, 32 * 128), np.float32)
    bcomb = np.empty((128, 32), np.float32)
    for m, (ci, pi, pj, al, be, cc, taps) in enumerate(_chunk_meta()):
        Wc = sum(Wout[:, t * 256 + cc * 128 : t * 256 + (cc + 1) * 128] for t in taps)
        bc = sum(bout[t * 256 + cc * 128 : t * 256 + (cc + 1) * 128] for t in taps)
        wcomb[:, m * 128 : (m + 1) * 128] = Wc
        bcomb[:, m] = bc

    # wm: w2 | w3 | w4ext | wcomb_ext  -> [33, 97 + 4096]; row 32 carries bcomb
    # (the kern matmul consumes an appended ones-row in h4, folding the bias
    # into the PE accumulation at zero cost)
    wm = np.zeros((33, 97 + 32 * 128), np.float32)
    wm[:32, 0:32] = np.asarray(inputs["W2"], np.float32)
    wm[:32, 32:64] = np.asarray(inputs["W3"], np.float32)
    # W4 gets a 33rd output column of zeros; with bias 1.0 it yields the
    # constant ones-row in h4 that carries bcomb through the kern matmul
    wm[:32, 64:96] = np.asarray(inputs["W4"], np.float32)
    wm[:32, 97:] = wcomb
    wm[32, 97:] = bcomb.T.reshape(-1)
    # bs: b1..b4 -> [33, 4]; bs[32, 3] = 1.0 feeds the h4 ones-row
    bs = np.zeros((33, 4), np.float32)
    for i in (1, 2, 3, 4):
        bs[:32, i - 1] = np.asarray(inputs[f"b{i}"], np.float32)
    bs[32, 3] = 1.0
    bb = np.asarray(inputs["b_proj"], np.float32).reshape(128, 1)
    wproj = np.ascontiguousarray(
        np.asarray(inputs["W_proj"], np.float32).reshape(2, 128, F_OUT).transpose(1, 0, 2)
    ).astype(BF16)  # [128c, 2cc, F]

    w1 = np.asarray(inputs["W1"], np.float32)  # [118, 32]
    in_maps = []
    for k in range(N_CORES):
        m0 = k * QR  # first source row of this core's slab
        slab = mp[:, m0 : m0 + MROWS, :, :]  # [B,10,66,C]
        x_cm = np.ascontiguousarray(slab.transpose(3, 0, 1, 2)).reshape(
            2, 128, B, MROWS, MCOLS
        ).astype(BF16)
        # feature columns grouped by parity class: (class, q, j); append W1
        r0 = k * RPC
        fs = feat[r0 : r0 + RPC]  # [16,128,118]
        fcls = np.stack(
            [fs[pi::2, pj::2].reshape(QR * QC, D_IN) for (pi, pj) in _CLASSES]
        )  # [4, 512, 118]
        fw1 = np.concatenate(
            [w1, np.ascontiguousarray(fcls.reshape(4 * QR * QC, D_IN).T)], axis=1
        )  # [118, 2080] = [w1 | feat]
        in_maps.append({"x": x_cm, "fw1": fw1, "wm": wm, "bs": bs, "bb": bb,
                        "wproj": wproj})
    return in_maps


def _gather(results):
    """results[k]["y"] [F, 4class, B, 512] -> [B, H_T, W_T, F] fp32."""
    out = np.empty((B, H_T, W_T, F_OUT), np.float32)
    for k, res in enumerate(results):
        y5 = res["y"].reshape(F_OUT, 4, B, QR, QC)
        slab = out[:, k * RPC : (k + 1) * RPC]  # [B,16,128,F] view
        for ci, (pi, pj) in enumerate(_CLASSES):
            slab[:, pi::2, pj::2] = y5[:, ci].transpose(1, 2, 3, 0)
    return out


# -------------------------------------------------------------- device program
def _build_program(repeat: int = 1, loop_repeat: int = 1):
    f32, bf16 = mybir.dt.float32, mybir.dt.bfloat16
    Relu = mybir.ActivationFunctionType.Relu
    Ident = mybir.ActivationFunctionType.Identity

    nc = bacc.Bacc("TRN2", target_bir_lowering=False, debug=False, num_devices=N_CORES)
    x_d = nc.dram_tensor("x", (2, 128, B, MROWS, MCOLS), bf16, kind="ExternalInput")
    fw1_d = nc.dram_tensor("fw1", (D_IN, NPIX + 32), F32R, kind="ExternalInput")
    wm_d = nc.dram_tensor("wm", (33, 97 + 32 * 128), F32R, kind="ExternalInput")
    bs_d = nc.dram_tensor("bs", (33, 4), f32, kind="ExternalInput")
    bb_d = nc.dram_tensor("bb", (128, 1), f32, kind="ExternalInput")
    wproj_d = nc.dram_tensor("wproj", (128, 2, F_OUT), bf16, kind="ExternalInput")
    y_d = nc.dram_tensor("y", (F_OUT, 4, B, 512), bf16, kind="ExternalOutput")

    meta = _chunk_meta()

    with tile.TileContext(nc) as tc:
        with (
            tc.tile_pool(name="const", bufs=1) as const,
            tc.tile_pool(name="hbuf", bufs=2) as hbuf,
            tc.tile_pool(name="kern", bufs=2) as kern_pool,
            tc.tile_pool(name="zbuf", bufs=12) as zbuf,
            tc.tile_pool(name="zpbuf", bufs=3) as zpbuf,
            tc.tile_pool(name="ybuf", bufs=2) as ybuf,
            # MLP and kern matmuls share one double-buffered 2-bank pool (4
            # banks); apply output gets the other 4 banks. MLP runs strictly
            # before the kern stage, so sharing costs no overlap.
            tc.tile_pool(name="ps_a", bufs=2, space="PSUM") as ps_a,
            tc.tile_pool(name="ps_y", bufs=2, space="PSUM") as ps_y,
        ):
            # ---- input loads, in need-order: MLP biases + w1 + class-0
            # features, MLP/kern weights, the image, remaining features,
            # projection weights ----
            bs_sb = const.tile([33, 4], f32)
            nc.sync.dma_start(bs_sb, bs_d[:])
            fw1_sb = const.tile([D_IN, NPIX + 32], F32R)
            nc.sync.dma_start(fw1_sb[:, 0:544], fw1_d[:, 0:544])
            wm_sb = const.tile([33, 97 + 32 * 128], F32R)
            nc.sync.dma_start(wm_sb, wm_d[:])
            x_sb = const.tile([128, 2, B, MROWS, MCOLS], bf16)
            nc.sync.dma_start(x_sb, x_d[:].transpose((1, 0, 2, 3, 4)))
            nc.sync.dma_start(fw1_sb[:, 544:], fw1_d[:, 544:])
            wproj_sb = const.tile([128, 2, F_OUT], bf16)
            nc.sync.dma_start(wproj_sb, wproj_d[:])
            bb_sb = const.tile([128, 1], f32)
            nc.sync.dma_start(bb_sb, bb_d[:])

            w_sb = {
                1: fw1_sb[:, 0:32],
                2: wm_sb[0:32, 0:32],
                3: wm_sb[0:32, 32:64],
                4: wm_sb[0:32, 64:97],
            }
            wcomb_sb = wm_sb[:, 97 : 97 + 32 * 128]

            def _body_all():
                reps = [c for _ in range(repeat) for c in range(4)]

                def mlp(ci):
                    h = fw1_sb[:, 32 + ci * 512 : 32 + (ci + 1) * 512]
                    for i in range(1, 5):
                        rows = 33 if i == 4 else 32
                        ps = ps_a.tile([rows, 512], f32, tag="A", name="ps")
                        nc.tensor.matmul(ps, w_sb[i], h, start=True, stop=True)
                        hn = hbuf.tile([rows, 512], F32R, tag=f"h{ci}", name="hn")
                        nc.scalar.activation(
                            hn, ps, Relu, bias=bs_sb[0:rows, i - 1 : i], scale=1.0
                        )
                        h = hn
                    return h

                def kerns(ci, h):
                    # one [128,1024] 2-bank psum + one ACT copy per A-pair
                    # (both c-halves share alpha/beta)
                    tiles = {}
                    for A in range(4):
                        m0 = ci * 8 + A * 2
                        _, _, _, alpha, beta, _, _ = meta[m0]
                        ps = ps_a.tile([128, 1024], f32, tag="A", name="ps")
                        for cc in range(2):
                            nc.tensor.matmul(
                                ps[:, cc * 512 : (cc + 1) * 512],
                                wcomb_sb[:, (m0 + cc) * 128 : (m0 + cc + 1) * 128],
                                h,
                                start=True,
                                stop=True,
                            )
                        ps28 = ps.rearrange("p (c a b) -> p c a b", c=2, a=QR)
                        wide = MCOLS if beta == 0 else QC
                        km = kern_pool.tile(
                            [128, 2, QR, wide], bf16, tag=f"k{A}", name="km"
                        )
                        if beta == 0:
                            border = bass.AP(
                                tensor=km.tensor,
                                offset=km.offset,
                                ap=[km.ap[0], km.ap[1], km.ap[2], [65, 2]],
                            )
                            nc.gpsimd.memset(border, 0.0)
                            nc.scalar.copy(km[:, :, :, 1:65], ps28)
                        else:
                            nc.scalar.copy(km, ps28)
                        for cc in range(2):
                            tiles[A * 2 + cc] = (km[:, cc], alpha, beta)
                    return tiles

                def apply(ci, kern_tiles):
                    # batch-broadcast multiplies feeding two 2-bank PSUM
                    # accumulators (batch pairs); the first pair's copy+DMA
                    # overlaps the second pair's matmul pass. The chunk the
                    # PE consumes last runs on the (otherwise idle) gpsimd
                    # engine, emitted first so its ~3us hides under the DVE
                    # stream.
                    # gpsimd takes the chunk whose km is drained FIRST (A=0)
                    # so its slow op starts early; the PE accumulation chain
                    # consumes it LAST (custom consume order below).
                    import os
                    zs = [None] * 8
                    pool_idx = int(os.environ.get("K_POOL_IDX", "4"))  # (cc=1, A=0); -1 disables
                    consume_order = [0, 1, 2, 3, 5, 6, 7, 4] if pool_idx == 4 else list(range(8))
                    for idx in ([pool_idx] if pool_idx >= 0 else []) + [i for i in range(8) if i != pool_idx]:
                        cc, A = idx // 4, idx % 4
                        km, alpha, beta = kern_tiles[A * 2 + cc]
                        kb = bass.AP(
                            tensor=km.tensor,
                            offset=km.offset,
                            ap=[km.ap[0], [0, B], *km.ap[1:]],
                        )
                        rows = slice(1 + alpha, 1 + alpha + QR)
                        eng = nc.gpsimd if idx == pool_idx else nc.vector
                        buf = zpbuf if idx == pool_idx else zbuf
                        tag = "zp" if idx == pool_idx else "z"
                        if beta == 0:
                            z = buf.tile([128, B, QR, MCOLS], bf16, tag=tag,
                                         name="z")
                            eng.tensor_mul(
                                z, x_sb[:, cc, :, rows, 0:MCOLS], kb
                            )
                            rhss = [z[:, b, :, 1:65] for b in range(B)]
                        else:
                            c0 = 1 + beta  # 0 or 2, 4B-aligned either way
                            z = buf.tile([128, B, QR, QC], bf16, tag=tag,
                                         name="z")
                            eng.tensor_mul(
                                z, x_sb[:, cc, :, rows, c0 : c0 + QC], kb
                            )
                            rhss = [z[:, b] for b in range(B)]
                        zs[idx] = rhss
                    for half in range(2):
                        yp = ps_y.tile([128, 1024], f32, tag="Y", name="yp")
                        for n, i in enumerate(consume_order):
                            rhss = zs[i]
                            for b2 in range(2):
                                nc.tensor.matmul(
                                    yp[:, b2 * 512 : (b2 + 1) * 512].rearrange(
                                        "p (a b) -> p a b", a=QR
                                    ),
                                    wproj_sb[:, (i // 4) % 2, :],
                                    rhss[half * 2 + b2],
                                    start=(n == 0),
                                    stop=(n == 7),
                                )
                        ys = ybuf.tile([F_OUT, 1024], bf16, tag="ysb", name="ys")
                        nc.scalar.activation(
                            ys, yp, Ident, bias=bb_sb[:, 0:1], scale=1.0
                        )
                        nc.sync.dma_start(
                            y_d[:, ci, half * 2 : half * 2 + 2],
                            ys.rearrange("p (a b) -> p a b", a=2),
                        )

                # 3-stage software pipeline: mlp two classes ahead, kern one
                # class ahead of its apply stage. PSUM tiles from the shared
                # mlp/kern pool are emitted in pipeline order so the pool's
                # buffer rotation follows the steady-state schedule instead
                # of serializing a monolithic upfront MLP phase.
                n = len(reps)
                hs, ks = {}, {}
                hs[0] = mlp(reps[0])
                if n > 1:
                    hs[1] = mlp(reps[1])
                ks[0] = kerns(reps[0], hs[0])
                for idx, ci in enumerate(reps):
                    if idx + 1 < n:
                        ks[idx + 1] = kerns(reps[idx + 1], hs[idx + 1])
                        del hs[idx + 1]
                    if idx + 2 < n:
                        hs[idx + 2] = mlp(reps[idx + 2])
                    apply(ci, ks.pop(idx))

            if loop_repeat > 1:
                with tc.For_i(
                    0, loop_repeat, 1,
                    hint_engines=(mybir.EngineType.PE, mybir.EngineType.Activation),
                ):
                    _body_all()
            else:
                _body_all()

    nc.compile()
    return nc


def get_program(repeat: int = 1, loop_repeat: int = 1):
    key = f"nc{repeat}_{loop_repeat}"
    nc = _CACHE.get(key)
    if nc is None:
        nc = _CACHE[key] = _build_program(repeat, loop_repeat)
    return nc


# --------------------------------------------------------------------- entry
def kernel(**inputs) -> np.ndarray:
    nc = get_program()
    in_maps = _host_prep(inputs)
    res = bass_utils.run_bass_kernel_spmd(
        nc, in_maps, core_ids=list(range(N_CORES))
    )
    return _gather(res.results)



# revision 20
# speedup vs baseline: 1.0289x; 1.0289x over previous
"""Trainium2 Bass kernel for nn_CUFLayer_83640193122985.

CUF layer: per-pixel hypernet MLP (118->32->32->32->32->2304) generates 3x3
per-channel kernels at each of 128x128 target pixels; applied to the 2x
nearest-upsampled main_input [4,64,64,256]; then 1x1 projection [256->128].

Key algebraic optimization (parity decomposition): the upsample is exactly 2x
nearest-neighbor, so each output pixel's 3x3 window covers only 2x2 DISTINCT
source pixels; which taps collapse onto which source pixel depends only on the
output pixel's (row, col) parity. W_out/b_out columns are pre-combined on the
host per parity class, turning 9 multiply-taps into 4 and letting the whole
apply stage run at source resolution.

Sharding: 8-way data parallel over output rows (16 rows/core, all batches),
hypernet recomputed per-core for its slab; no collectives. The DCT feature
matrix is input-independent and precomputed on host. Matmuls run in float32r
(near-fp32 precision at full PE rate); the per-pixel multiply runs in bf16 on
the vector engine (2x packed mode, batch-broadcast); tap and channel
accumulation ride the PE's PSUM accumulation fused with the 1x1 projection.
Inputs are packed into few DRAM tensors in need-order (HWDGE dispatch is
~0.65us/DMA, transfers serialize at ~360GB/s); per-class outputs accumulate in
two 2-bank PSUM batch-pair tiles whose copy+DMA overlap each other's matmuls.

Self-contained: hardcodes all shapes; no sibling imports.
"""

import numpy as np
import ml_dtypes

import concourse.bass as bass
import concourse.mybir as mybir
import concourse.tile as tile
from concourse import bacc
from concourse import bass_utils

BF16 = ml_dtypes.bfloat16
F32R = mybir.dt.float32r

K = 3
DCT_BASIS = 25
B, H_IN, W_IN, C = 4, 64, 64, 256
H_T, W_T, F_OUT = 128, 128, 128
N_CORES = 8
RPC = H_T // N_CORES  # 16 output rows per core
D_IN = 118
NPIX = RPC * W_T  # 2048 pixels per core
MROWS = RPC // 2 + 2  # 10 source rows incl halo
MCOLS = W_IN + 2  # 66 source cols incl halo
QR = RPC // 2  # 8 source-row positions per core
QC = W_IN  # 64 source-col positions

# vertical tap-collapse table: V[pi][ai] = (alpha, [di...]); same for cols
_V = {0: [(-1, [0]), (0, [1, 2])], 1: [(0, [0, 1]), (1, [2])]}
_CLASSES = [(0, 0), (0, 1), (1, 0), (1, 1)]

_CACHE: dict = {}


# ----------------------------------------------------------------- host side
def _build_features():
    """feat [H_T, W_T, 118] fp32 — input-independent constant."""
    f = np.linspace(1.0, 2.0, DCT_BASIS).astype(np.float32)
    gh = np.linspace(0.0, 1.0, H_T).astype(np.float32)
    row_enc = np.cos(np.pi * (2.0 * gh[:, None] + 1.0) * f[None, :]).astype(np.float32)
    delta = np.concatenate(
        [
            np.broadcast_to(row_enc[:, None, :], (H_T, W_T, DCT_BASIS)),
            np.broadcast_to(row_enc[None, :, :], (H_T, W_T, DCT_BASIS)),
        ],
        axis=-1,
    )
    scale = np.array([H_T / H_IN, W_T / W_IN], np.float32)
    scale_enc = np.cos(np.pi * (2.0 * scale[:, None] + 1.0) * f[None, :]).reshape(-1)
    offs = np.arange(K, dtype=np.float32) - 1.0
    ki, kj = np.meshgrid(offs, offs, indexing="ij")
    kidx = np.stack([ki, kj], -1).reshape(K * K, 2)
    f9 = np.linspace(1.0, 1.0, 9).astype(np.float32)
    kenc = np.cos(np.pi * (2.0 * kidx[..., None] + 1.0) * f9).reshape(K * K, 18).mean(0)
    feat = np.concatenate(
        [
            delta,
            np.broadcast_to(scale_enc, (H_T, W_T, 50)),
            np.broadcast_to(kenc.astype(np.float32), (H_T, W_T, 18)),
        ],
        axis=-1,
    ).astype(np.float32)
    return feat  # [128,128,118]


def _chunk_meta():
    """Per combined-kernel chunk m = class*8 + A*2 + cc: (class, pi, pj,
    alpha, beta, cc, taps). A = ai*2 + bi."""
    meta = []
    for ci, (pi, pj) in enumerate(_CLASSES):
        for ai in range(2):
            for bi in range(2):
                alpha, dis = _V[pi][ai]
                beta, djs = _V[pj][bi]
                taps = [di * 3 + dj for di in dis for dj in djs]
                for cc in range(2):
                    meta.append((ci, pi, pj, alpha, beta, cc, taps))
    return meta


def _host_prep(inputs):
    """Build per-core input maps (few, large tensors to minimize DMA count)."""
    main_input = np.asarray(inputs["main_input"], np.float32)
    feat = _CACHE.get("feat")
    if feat is None:
        feat = _CACHE["feat"] = _build_features()

    # source image, zero-padded by 1: [B, 66, 66, C] then channel-major bf16
    mp = np.pad(main_input, ((0, 0), (1, 1), (1, 1), (0, 0)))

    Wout = np.asarray(inputs["W_out"], np.float32)  # [32, 2304] cols t*256+c
    bout = np.asarray(inputs["b_out"], np.float32)
    wcomb = np.empty((32, 32 * 128), np.float32)
    bcomb = np.empty((128, 32), np.float32)
    for m, (ci, pi, pj, al, be, cc, taps) in enumerate(_chunk_meta()):
        Wc = sum(Wout[:, t * 256 + cc * 128 : t * 256 + (cc + 1) * 128] for t in taps)
        bc = sum(bout[t * 256 + cc * 128 : t * 256 + (cc + 1) * 128] for t in taps)
        wcomb[:, m * 128 : (m + 1) * 128] = Wc
        bcomb[:, m] = bc

    # wm: w2 | w3 | w4ext | wcomb_ext  -> [33, 97 + 4096]; row 32 carries bcomb
    # (the kern matmul consumes an appended ones-row in h4, folding the bias
    # into the PE accumulation at zero cost)
    wm = np.zeros((33, 97 + 32 * 128), np.float32)
    wm[:32, 0:32] = np.asarray(inputs["W2"], np.float32)
    wm[:32, 32:64] = np.asarray(inputs["W3"], np.float32)
    # W4 gets a 33rd output column of zeros; with bias 1.0 it yields the
    # constant ones-row in h4 that carries bcomb through the kern matmul
    wm[:32, 64:96] = np.asarray(inputs["W4"], np.float32)
    wm[:32, 97:] = wcomb
    wm[32, 97:] = bcomb.T.reshape(-1)
    # bs: b1..b4 -> [33, 4]; bs[32, 3] = 1.0 feeds the h4 ones-row
    bs = np.zeros((33, 4), np.float32)
    for i in (1, 2, 3, 4):
        bs[:32, i - 1] = np.asarray(inputs[f"b{i}"], np.float32)
    bs[32, 3] = 1.0
    bb = np.asarray(inputs["b_proj"], np.float32).reshape(128, 1)
    wproj = np.ascontiguousarray(
        np.asarray(inputs["W_proj"], np.float32).reshape(2, 128, F_OUT).transpose(1, 0, 2)
    ).astype(BF16)  # [128c, 2cc, F]

    w1 = np.asarray(inputs["W1"], np.float32)  # [118, 32]
    in_maps = []
    for k in range(N_CORES):
        m0 = k * QR  # first source row of this core's slab
        slab = mp[:, m0 : m0 + MROWS, :, :]  # [B,10,66,C]
        x_cm = np.ascontiguousarray(slab.transpose(3, 0, 1, 2)).reshape(
            2, 128, B, MROWS, MCOLS
        ).astype(BF16)
        # feature columns grouped by parity class: (class, q, j); append W1
        r0 = k * RPC
        fs = feat[r0 : r0 + RPC]  # [16,128,118]
        fcls = np.stack(
            [fs[pi::2, pj::2].reshape(QR * QC, D_IN) for (pi, pj) in _CLASSES]
        )  # [4, 512, 118]
        fw1 = np.concatenate(
            [w1, np.ascontiguousarray(fcls.reshape(4 * QR * QC, D_IN).T)], axis=1
        )  # [118, 2080] = [w1 | feat]
        in_maps.append({"x": x_cm, "fw1": fw1, "wm": wm, "bs": bs, "bb": bb,
                        "wproj": wproj})
    return in_maps


def _gather(results):
    """results[k]["y"] [F, 4class, B, 512] -> [B, H_T, W_T, F] fp32."""
    out = np.empty((B, H_T, W_T, F_OUT), np.float32)
    for k, res in enumerate(results):
        y5 = res["y"].reshape(F_OUT, 4, B, QR, QC)
        slab = out[:, k * RPC : (k + 1) * RPC]  # [B,16,128,F] view
        for ci, (pi, pj) in enumerate(_CLASSES):
            slab[:, pi::2, pj::2] = y5[:, ci].transpose(1, 2, 3, 0)
    return out


# -------------------------------------------------------------- device program
def _build_program(repeat: int = 1, loop_repeat: int = 1):
    f32, bf16 = mybir.dt.float32, mybir.dt.bfloat16
    Relu = mybir.ActivationFunctionType.Relu
    Ident = mybir.ActivationFunctionType.Identity

    nc = bacc.Bacc("TRN2", target_bir_lowering=False, debug=False, num_devices=N_CORES)
    x_d = nc.dram_tensor("x", (2, 128, B, MROWS, MCOLS), bf16, kind="ExternalInput")
    fw1_d = nc.dram_tensor("fw1", (D_IN, NPIX + 32), F32R, kind="ExternalInput")
    wm_d = nc.dram_tensor("wm", (33, 97 + 32 * 128), F32R, kind="ExternalInput")
    bs_d = nc.dram_tensor("bs", (33, 4), f32, kind="ExternalInput")
    bb_d = nc.dram_tensor("bb", (128, 1), f32, kind="ExternalInput")
    wproj_d = nc.dram_tensor("wproj", (128, 2, F_OUT), bf16, kind="ExternalInput")
    y_d = nc.dram_tensor("y", (F_OUT, 4, B, 512), bf16, kind="ExternalOutput")

    meta = _chunk_meta()

    with tile.TileContext(nc) as tc:
        with (
            tc.tile_pool(name="const", bufs=1) as const,
            tc.tile_pool(name="hbuf", bufs=2) as hbuf,
            tc.tile_pool(name="kern", bufs=3) as kern_pool,
            tc.tile_pool(name="zbuf", bufs=12) as zbuf,
            tc.tile_pool(name="zpbuf", bufs=3) as zpbuf,
            tc.tile_pool(name="ybuf", bufs=2) as ybuf,
            # 8 PSUM banks: mlp 2x1 + kern 2x2 + y 2x1. Decoupled pools keep
            # the mlp chain, kern production, and apply output from
            # serializing each other through buffer rotation.
            tc.tile_pool(name="ps_mlp", bufs=2, space="PSUM") as ps_mlp,
            tc.tile_pool(name="ps_kern", bufs=2, space="PSUM") as ps_kern,
            tc.tile_pool(name="ps_y", bufs=2, space="PSUM") as ps_y,
        ):
            # ---- input loads, in need-order: MLP biases + w1 + class-0
            # features, MLP/kern weights, the image, remaining features,
            # projection weights ----
            bs_sb = const.tile([33, 4], f32)
            nc.sync.dma_start(bs_sb, bs_d[:])
            fw1_sb = const.tile([D_IN, NPIX + 32], F32R)
            nc.sync.dma_start(fw1_sb[:, 0:544], fw1_d[:, 0:544])
            wm_sb = const.tile([33, 97 + 32 * 128], F32R)
            nc.sync.dma_start(wm_sb, wm_d[:])
            x_sb = const.tile([128, 2, B, MROWS, MCOLS], bf16)
            nc.sync.dma_start(x_sb, x_d[:].transpose((1, 0, 2, 3, 4)))
            nc.sync.dma_start(fw1_sb[:, 544:], fw1_d[:, 544:])
            wproj_sb = const.tile([128, 2, F_OUT], bf16)
            nc.sync.dma_start(wproj_sb, wproj_d[:])
            bb_sb = const.tile([128, 1], f32)
            nc.sync.dma_start(bb_sb, bb_d[:])

            w_sb = {
                1: fw1_sb[:, 0:32],
                2: wm_sb[0:32, 0:32],
                3: wm_sb[0:32, 32:64],
                4: wm_sb[0:32, 64:97],
            }
            wcomb_sb = wm_sb[:, 97 : 97 + 32 * 128]

            def _body_all():
                reps = [c for _ in range(repeat) for c in range(4)]

                def mlp(ci):
                    h = fw1_sb[:, 32 + ci * 512 : 32 + (ci + 1) * 512]
                    for i in range(1, 5):
                        rows = 33 if i == 4 else 32
                        ps = ps_mlp.tile([rows, 512], f32, tag="mlp", name="ps")
                        nc.tensor.matmul(ps, w_sb[i], h, start=True, stop=True)
                        hn = hbuf.tile([rows, 512], F32R, tag=f"h{ci}", name="hn")
                        nc.scalar.activation(
                            hn, ps, Relu, bias=bs_sb[0:rows, i - 1 : i], scale=1.0
                        )
                        h = hn
                    return h

                def kerns(ci, h):
                    # one [128,1024] 2-bank psum + one ACT copy per A-pair
                    # (both c-halves share alpha/beta)
                    import os
                    if os.environ.get("K_BISECT", "") == "hconst":
                        h = wm_sb[0:33, 0:512]  # cut the MLP->kern edge
                    tiles = {}
                    for A in range(4):
                        m0 = ci * 8 + A * 2
                        _, _, _, alpha, beta, _, _ = meta[m0]
                        ps = ps_kern.tile([128, 1024], f32, tag="kps", name="ps")
                        for cc in range(2):
                            nc.tensor.matmul(
                                ps[:, cc * 512 : (cc + 1) * 512],
                                wcomb_sb[:, (m0 + cc) * 128 : (m0 + cc + 1) * 128],
                                h,
                                start=True,
                                stop=True,
                            )
                        ps28 = ps.rearrange("p (c a b) -> p c a b", c=2, a=QR)
                        wide = MCOLS if beta == 0 else QC
                        km = kern_pool.tile(
                            [128, 2, QR, wide], bf16, tag=f"k{A}", name="km"
                        )
                        if beta == 0:
                            border = bass.AP(
                                tensor=km.tensor,
                                offset=km.offset,
                                ap=[km.ap[0], km.ap[1], km.ap[2], [65, 2]],
                            )
                            nc.gpsimd.memset(border, 0.0)
                            nc.scalar.copy(km[:, :, :, 1:65], ps28)
                        else:
                            nc.scalar.copy(km, ps28)
                        for cc in range(2):
                            tiles[A * 2 + cc] = (km[:, cc], alpha, beta)
                    return tiles

                def apply(ci, kern_tiles):
                    # batch-broadcast multiplies feeding two 2-bank PSUM
                    # accumulators (batch pairs); the first pair's copy+DMA
                    # overlaps the second pair's matmul pass. The chunk the
                    # PE consumes last runs on the (otherwise idle) gpsimd
                    # engine, emitted first so its ~3us hides under the DVE
                    # stream.
                    # gpsimd takes the chunk whose km is drained FIRST (A=0)
                    # so its slow op starts early; the PE accumulation chain
                    # consumes it LAST (custom consume order below).
                    import os
                    zs = [None] * 8
                    pool_idx = int(os.environ.get("K_POOL_IDX", "-1"))  # (cc=1, A=0); -1 disables
                    bisect = os.environ.get("K_BISECT", "")
                    # A-major order: each km drain (per A) immediately enables
                    # the next two multiplies, minimizing drain->DVE stalls
                    consume_order = [0, 4, 1, 5, 2, 6, 3, 7]
                    for idx in ([pool_idx] if pool_idx >= 0 else []) + [i for i in consume_order if i != pool_idx]:
                        cc, A = idx // 4, idx % 4
                        km, alpha, beta = kern_tiles[A * 2 + cc]
                        kb = bass.AP(
                            tensor=km.tensor,
                            offset=km.offset,
                            ap=[km.ap[0], [0, B], *km.ap[1:]],
                        )
                        rows = slice(1 + alpha, 1 + alpha + QR)
                        eng = nc.gpsimd if idx == pool_idx else nc.vector
                        buf = zpbuf if idx == pool_idx else zbuf
                        tag = "zp" if idx == pool_idx else "z"
                        if bisect == "kconst":
                            # cut the ACT(km)->DVE edge: multiply x by itself
                            kb = None
                        if beta == 0:
                            z = buf.tile([128, B, QR, MCOLS], bf16, tag=tag,
                                         name="z")
                            xin = x_sb[:, cc, :, rows, 0:MCOLS]
                            eng.tensor_mul(z, xin, kb if kb is not None else xin)
                            rhss = [z[:, b, :, 1:65] for b in range(B)]
                        else:
                            c0 = 1 + beta  # 0 or 2, 4B-aligned either way
                            z = buf.tile([128, B, QR, QC], bf16, tag=tag,
                                         name="z")
                            xin = x_sb[:, cc, :, rows, c0 : c0 + QC]
                            eng.tensor_mul(z, xin, kb if kb is not None else xin)
                            rhss = [z[:, b] for b in range(B)]
                        if bisect == "zconst":
                            # cut the DVE(z)->PE edge: matmuls read x directly
                            rhss = [x_sb[:, cc, b, 0:QR, 1:65] for b in range(B)]
                        zs[idx] = rhss
                    # four 8-matmul chains (one per batch) into 1-bank PSUM
                    # tiles; ACT assembles all four into one SBUF tile, one
                    # DMA per class
                    ys = ybuf.tile([F_OUT, 4, 512], bf16, tag="ysb", name="ys")
                    for half in range(2):
                        for b2 in range(2):
                            b = half * 2 + b2
                            yp = ps_y.tile([128, 512], f32, tag="Y", name="yp")
                            for n, i in enumerate(consume_order):
                                nc.tensor.matmul(
                                    yp.rearrange("p (a b) -> p a b", a=QR),
                                    wproj_sb[:, (i // 4) % 2, :],
                                    zs[i][b],
                                    start=(n == 0),
                                    stop=(n == 7),
                                )
                            nc.scalar.activation(
                                ys[:, b], yp, Ident, bias=bb_sb[:, 0:1],
                                scale=1.0,
                            )
                    nc.sync.dma_start(y_d[:, ci], ys)

                # deep software pipeline: mlp three classes ahead, kern two
                # classes ahead of the apply stage, so the DVE multiply
                # stream never waits on kern drains and classes overlap.
                n = len(reps)
                hs, ks = {}, {}
                for j in range(min(3, n)):
                    hs[j] = mlp(reps[j])
                for j in range(min(2, n)):
                    ks[j] = kerns(reps[j], hs[j])
                for idx, ci in enumerate(reps):
                    if idx + 2 < n:
                        ks[idx + 2] = kerns(reps[idx + 2], hs[idx + 2])
                    if idx + 3 < n:
                        hs[idx + 3] = mlp(reps[idx + 3])
                    apply(ci, ks.pop(idx))

            if loop_repeat > 1:
                with tc.For_i(
                    0, loop_repeat, 1,
                    hint_engines=(mybir.EngineType.PE, mybir.EngineType.Activation),
                ):
                    _body_all()
            else:
                _body_all()

    nc.compile()
    return nc


def get_program(repeat: int = 1, loop_repeat: int = 1):
    key = f"nc{repeat}_{loop_repeat}"
    nc = _CACHE.get(key)
    if nc is None:
        nc = _CACHE[key] = _build_program(repeat, loop_repeat)
    return nc


# --------------------------------------------------------------------- entry
def kernel(**inputs) -> np.ndarray:
    nc = get_program()
    in_maps = _host_prep(inputs)
    res = bass_utils.run_bass_kernel_spmd(
        nc, in_maps, core_ids=list(range(N_CORES))
    )
    return _gather(res.results)



# revision 34
# speedup vs baseline: 1.1427x; 1.1106x over previous
"""Trainium2 Bass kernel for nn_CUFLayer_83640193122985.

CUF layer: per-pixel hypernet MLP (118->32->32->32->32->2304) generates 3x3
per-channel kernels at each of 128x128 target pixels; applied to the 2x
nearest-upsampled main_input [4,64,64,256]; then 1x1 projection [256->128].

Key algebraic optimization (parity decomposition): the upsample is exactly 2x
nearest-neighbor, so each output pixel's 3x3 window covers only 2x2 DISTINCT
source pixels; which taps collapse onto which source pixel depends only on the
output pixel's (row, col) parity. W_out/b_out columns are pre-combined on the
host per parity class, turning 9 multiply-taps into 4 and letting the whole
apply stage run at source resolution.

Sharding: 8-way data parallel over output rows (16 rows/core, all batches),
hypernet recomputed per-core for its slab; no collectives. The DCT feature
matrix is input-independent and precomputed on host. Matmuls run in float32r
(near-fp32 precision at full PE rate); the per-pixel multiply runs in bf16 on
the vector engine (2x packed mode, batch-broadcast); tap and channel
accumulation ride the PE's PSUM accumulation fused with the 1x1 projection.
Inputs are packed into few DRAM tensors in need-order (HWDGE dispatch is
~0.65us/DMA, transfers serialize at ~360GB/s); per-class outputs accumulate in
two 2-bank PSUM batch-pair tiles whose copy+DMA overlap each other's matmuls.

Self-contained: hardcodes all shapes; no sibling imports.
"""

import numpy as np
import ml_dtypes

import concourse.bass as bass
import concourse.mybir as mybir
import concourse.tile as tile
from concourse import bacc
from concourse import bass_utils

BF16 = ml_dtypes.bfloat16
F32R = mybir.dt.float32r
F16 = mybir.dt.float16

K = 3
DCT_BASIS = 25
B, H_IN, W_IN, C = 4, 64, 64, 256
H_T, W_T, F_OUT = 128, 128, 128
N_CORES = 8
RPC = H_T // N_CORES  # 16 output rows per core
D_IN = 118
NPIX = RPC * W_T  # 2048 pixels per core
MROWS = RPC // 2 + 2  # 10 source rows incl halo
MCOLS = W_IN + 2  # 66 source cols incl halo
QR = RPC // 2  # 8 source-row positions per core
QC = W_IN  # 64 source-col positions

# vertical tap-collapse table: V[pi][ai] = (alpha, [di...]); same for cols
_V = {0: [(-1, [0]), (0, [1, 2])], 1: [(0, [0, 1]), (1, [2])]}
_CLASSES = [(0, 0), (0, 1), (1, 0), (1, 1)]

_CACHE: dict = {}


# ----------------------------------------------------------------- host side
def _build_features():
    """feat [H_T, W_T, 118] fp32 — input-independent constant."""
    f = np.linspace(1.0, 2.0, DCT_BASIS).astype(np.float32)
    gh = np.linspace(0.0, 1.0, H_T).astype(np.float32)
    row_enc = np.cos(np.pi * (2.0 * gh[:, None] + 1.0) * f[None, :]).astype(np.float32)
    delta = np.concatenate(
        [
            np.broadcast_to(row_enc[:, None, :], (H_T, W_T, DCT_BASIS)),
            np.broadcast_to(row_enc[None, :, :], (H_T, W_T, DCT_BASIS)),
        ],
        axis=-1,
    )
    scale = np.array([H_T / H_IN, W_T / W_IN], np.float32)
    scale_enc = np.cos(np.pi * (2.0 * scale[:, None] + 1.0) * f[None, :]).reshape(-1)
    offs = np.arange(K, dtype=np.float32) - 1.0
    ki, kj = np.meshgrid(offs, offs, indexing="ij")
    kidx = np.stack([ki, kj], -1).reshape(K * K, 2)
    f9 = np.linspace(1.0, 1.0, 9).astype(np.float32)
    kenc = np.cos(np.pi * (2.0 * kidx[..., None] + 1.0) * f9).reshape(K * K, 18).mean(0)
    feat = np.concatenate(
        [
            delta,
            np.broadcast_to(scale_enc, (H_T, W_T, 50)),
            np.broadcast_to(kenc.astype(np.float32), (H_T, W_T, 18)),
        ],
        axis=-1,
    ).astype(np.float32)
    return feat  # [128,128,118]


def _chunk_meta():
    """Per combined-kernel chunk m = class*8 + A*2 + cc: (class, pi, pj,
    alpha, beta, cc, taps). A = ai*2 + bi."""
    meta = []
    for ci, (pi, pj) in enumerate(_CLASSES):
        for ai in range(2):
            for bi in range(2):
                alpha, dis = _V[pi][ai]
                beta, djs = _V[pj][bi]
                taps = [di * 3 + dj for di in dis for dj in djs]
                for cc in range(2):
                    meta.append((ci, pi, pj, alpha, beta, cc, taps))
    return meta


def _host_prep(inputs):
    """Build per-core input maps (few, large tensors to minimize DMA count)."""
    main_input = np.asarray(inputs["main_input"], np.float32)
    feat = _CACHE.get("feat")
    if feat is None:
        feat = _CACHE["feat"] = _build_features()

    # source image, zero-padded by 1: [B, 66, 66, C] then channel-major bf16
    mp = np.pad(main_input, ((0, 0), (1, 1), (1, 1), (0, 0)))

    Wout = np.asarray(inputs["W_out"], np.float32)  # [32, 2304] cols t*256+c
    bout = np.asarray(inputs["b_out"], np.float32)
    wcomb = np.empty((32, 32 * 128), np.float32)
    bcomb = np.empty((128, 32), np.float32)
    for m, (ci, pi, pj, al, be, cc, taps) in enumerate(_chunk_meta()):
        Wc = sum(Wout[:, t * 256 + cc * 128 : t * 256 + (cc + 1) * 128] for t in taps)
        bc = sum(bout[t * 256 + cc * 128 : t * 256 + (cc + 1) * 128] for t in taps)
        wcomb[:, m * 128 : (m + 1) * 128] = Wc
        bcomb[:, m] = bc

    # wm: w2 | w3 | w4ext | wcomb_ext  -> [33, 97 + 4096]; row 32 carries bcomb
    # (the kern matmul consumes an appended ones-row in h4, folding the bias
    # into the PE accumulation at zero cost)
    wm = np.zeros((33, 97 + 32 * 128), np.float32)
    wm[:32, 0:32] = np.asarray(inputs["W2"], np.float32)
    wm[:32, 32:64] = np.asarray(inputs["W3"], np.float32)
    # W4 gets a 33rd output column of zeros; with bias 1.0 it yields the
    # constant ones-row in h4 that carries bcomb through the kern matmul
    wm[:32, 64:96] = np.asarray(inputs["W4"], np.float32)
    wm[:32, 97:] = wcomb
    wm[32, 97:] = bcomb.T.reshape(-1)
    # bs: b1..b4 -> [33, 4]; bs[32, 3] = 1.0 feeds the h4 ones-row
    bs = np.zeros((33, 4), np.float32)
    for i in (1, 2, 3, 4):
        bs[:32, i - 1] = np.asarray(inputs[f"b{i}"], np.float32)
    bs[32, 3] = 1.0
    bb = np.asarray(inputs["b_proj"], np.float32).reshape(128, 1)
    wproj = np.ascontiguousarray(
        np.asarray(inputs["W_proj"], np.float32).reshape(2, 128, F_OUT).transpose(1, 0, 2)
    ).astype(BF16)  # [128c, 2cc, F]

    w1 = np.asarray(inputs["W1"], np.float32)  # [118, 32]
    in_maps = []
    for k in range(N_CORES):
        m0 = k * QR  # first source row of this core's slab
        slab = mp[:, m0 : m0 + MROWS, :, :]  # [B,10,66,C]
        x_cm = np.ascontiguousarray(slab.transpose(3, 0, 1, 2)).reshape(
            2, 128, B, MROWS, MCOLS
        ).astype(BF16)
        # feature columns grouped by parity class: (class, q, j); append W1
        r0 = k * RPC
        fs = feat[r0 : r0 + RPC]  # [16,128,118]
        fcls = np.stack(
            [fs[pi::2, pj::2].reshape(QR * QC, D_IN) for (pi, pj) in _CLASSES]
        )  # [4, 512, 118]
        fw1 = np.concatenate(
            [w1, np.ascontiguousarray(fcls.reshape(4 * QR * QC, D_IN).T)], axis=1
        ).astype(np.float16)  # [118, 2080] = [w1 | feat]
        in_maps.append({"x": x_cm, "fw1": fw1, "wm": wm.astype(np.float16),
                        "bs": bs, "bb": bb, "wproj": wproj})
    return in_maps


def _gather(results):
    """results[k]["y"] [F, 4class, B, 512] -> [B, H_T, W_T, F] fp32."""
    out = np.empty((B, H_T, W_T, F_OUT), np.float32)
    for k, res in enumerate(results):
        y5 = res["y"].reshape(F_OUT, 4, B, QR, QC)
        slab = out[:, k * RPC : (k + 1) * RPC]  # [B,16,128,F] view
        for ci, (pi, pj) in enumerate(_CLASSES):
            slab[:, pi::2, pj::2] = y5[:, ci].transpose(1, 2, 3, 0)
    return out


# -------------------------------------------------------------- device program
def _build_program(repeat: int = 1, loop_repeat: int = 1):
    f32, bf16 = mybir.dt.float32, mybir.dt.bfloat16
    Relu = mybir.ActivationFunctionType.Relu
    Ident = mybir.ActivationFunctionType.Identity

    nc = bacc.Bacc("TRN2", target_bir_lowering=False, debug=False, num_devices=N_CORES)
    x_d = nc.dram_tensor("x", (2, 128, B, MROWS, MCOLS), bf16, kind="ExternalInput")
    fw1_d = nc.dram_tensor("fw1", (D_IN, NPIX + 32), F16, kind="ExternalInput")
    wm_d = nc.dram_tensor("wm", (33, 97 + 32 * 128), F16, kind="ExternalInput")
    bs_d = nc.dram_tensor("bs", (33, 4), f32, kind="ExternalInput")
    bb_d = nc.dram_tensor("bb", (128, 1), f32, kind="ExternalInput")
    wproj_d = nc.dram_tensor("wproj", (128, 2, F_OUT), bf16, kind="ExternalInput")
    y_d = nc.dram_tensor("y", (F_OUT, 4, B, 512), bf16, kind="ExternalOutput")

    meta = _chunk_meta()

    with tile.TileContext(nc) as tc:
        with (
            tc.tile_pool(name="const", bufs=1) as const,
            tc.tile_pool(name="hbuf", bufs=2) as hbuf,
            tc.tile_pool(name="kern", bufs=3) as kern_pool,
            tc.tile_pool(name="zbuf", bufs=12) as zbuf,
            tc.tile_pool(name="zpbuf", bufs=3) as zpbuf,
            tc.tile_pool(name="ybuf", bufs=2) as ybuf,
            # 8 PSUM banks: mlp 2x1 + kern 2x2 + y 2x1. Decoupled pools keep
            # the mlp chain, kern production, and apply output from
            # serializing each other through buffer rotation.
            tc.tile_pool(name="ps_mlp", bufs=2, space="PSUM") as ps_mlp,
            tc.tile_pool(name="ps_kern", bufs=2, space="PSUM") as ps_kern,
            tc.tile_pool(name="ps_y", bufs=2, space="PSUM") as ps_y,
        ):
            # ---- input loads, in need-order: MLP biases + w1 + class-0
            # features, MLP/kern weights, the image, remaining features,
            # projection weights ----
            bs_sb = const.tile([33, 4], f32)
            nc.sync.dma_start(bs_sb, bs_d[:])
            fw1_sb = const.tile([D_IN, NPIX + 32], F16)
            nc.sync.dma_start(fw1_sb[:, 0:544], fw1_d[:, 0:544])
            wm_sb = const.tile([33, 97 + 32 * 128], F16)
            nc.sync.dma_start(wm_sb, wm_d[:])
            x_sb = const.tile([128, 2, B, MROWS, MCOLS], bf16)
            nc.sync.dma_start(x_sb, x_d[:].transpose((1, 0, 2, 3, 4)))
            nc.sync.dma_start(fw1_sb[:, 544:], fw1_d[:, 544:])
            wproj_sb = const.tile([128, 2, F_OUT], bf16)
            nc.sync.dma_start(wproj_sb, wproj_d[:])
            bb_sb = const.tile([128, 1], f32)
            nc.sync.dma_start(bb_sb, bb_d[:])

            w_sb = {
                1: fw1_sb[:, 0:32],
                2: wm_sb[0:32, 0:32],
                3: wm_sb[0:32, 32:64],
                4: wm_sb[0:32, 64:97],
            }
            wcomb_sb = wm_sb[:, 97 : 97 + 32 * 128]

            def _body_all():
                import os
                stage = int(os.environ.get("K_STAGE", "4"))
                # 1: DVE multiplies only (const k); 2: +apply/ys/DMA;
                # 3: +kern matmuls/drains (const h); 4: full; 5: full, no DMA
                reps = [c for _ in range(repeat) for c in range(4)]

                def mlp(ci):
                    h = fw1_sb[:, 32 + ci * 512 : 32 + (ci + 1) * 512]
                    for i in range(1, 5):
                        rows = 33 if i == 4 else 32
                        ps = ps_mlp.tile([rows, 512], f32, tag="mlp", name="ps")
                        nc.tensor.matmul(ps, w_sb[i], h, start=True, stop=True)
                        hn = hbuf.tile([rows, 512], F16, tag=f"h{ci}", name="hn")
                        nc.scalar.activation(
                            hn, ps, Relu, bias=bs_sb[0:rows, i - 1 : i], scale=1.0
                        )
                        h = hn
                    return h

                def kerns(ci, h):
                    # one [128,1024] 2-bank psum + one ACT copy per A-pair
                    # (both c-halves share alpha/beta)
                    if os.environ.get("K_BISECT", "") == "hconst" or stage == 3:
                        h = wm_sb[0:33, 0:512]  # cut the MLP->kern edge
                    if stage <= 2:
                        return {
                            A * 2 + cc: (None, meta[ci * 8 + A * 2][3],
                                         meta[ci * 8 + A * 2][4])
                            for A in range(4) for cc in range(2)
                        }
                    tiles = {}
                    for A in range(4):
                        m0 = ci * 8 + A * 2
                        _, _, _, alpha, beta, _, _ = meta[m0]
                        ps = ps_kern.tile([128, 1024], f32, tag="kps", name="ps")
                        for cc in range(2):
                            nc.tensor.matmul(
                                ps[:, cc * 512 : (cc + 1) * 512],
                                wcomb_sb[:, (m0 + cc) * 128 : (m0 + cc + 1) * 128],
                                h,
                                start=True,
                                stop=True,
                            )
                        ps28 = ps.rearrange("p (c a b) -> p c a b", c=2, a=QR)
                        wide = MCOLS if beta == 0 else QC
                        km = kern_pool.tile(
                            [128, 2, QR, wide], bf16, tag=f"k{A}", name="km"
                        )
                        if beta == 0:
                            border = bass.AP(
                                tensor=km.tensor,
                                offset=km.offset,
                                ap=[km.ap[0], km.ap[1], km.ap[2], [65, 2]],
                            )
                            nc.gpsimd.memset(border, 0.0)
                            nc.scalar.copy(km[:, :, :, 1:65], ps28)
                        else:
                            nc.scalar.copy(km, ps28)
                        for cc in range(2):
                            tiles[A * 2 + cc] = (km[:, cc], alpha, beta)
                    return tiles

                def apply(ci, kern_tiles):
                    # batch-broadcast multiplies feeding two 2-bank PSUM
                    # accumulators (batch pairs); the first pair's copy+DMA
                    # overlaps the second pair's matmul pass. The chunk the
                    # PE consumes last runs on the (otherwise idle) gpsimd
                    # engine, emitted first so its ~3us hides under the DVE
                    # stream.
                    # gpsimd takes the chunk whose km is drained FIRST (A=0)
                    # so its slow op starts early; the PE accumulation chain
                    # consumes it LAST (custom consume order below).
                    zs = [None] * 8
                    pool_idx = int(os.environ.get("K_POOL_IDX", "-1"))  # (cc=1, A=0); -1 disables
                    bisect = os.environ.get("K_BISECT", "")
                    # A-major order: each km drain (per A) immediately enables
                    # the next two multiplies, minimizing drain->DVE stalls
                    consume_order = [0, 4, 1, 5, 2, 6, 3, 7]
                    for idx in ([pool_idx] if pool_idx >= 0 else []) + [i for i in consume_order if i != pool_idx]:
                        cc, A = idx // 4, idx % 4
                        km, alpha, beta = kern_tiles[A * 2 + cc]
                        kb = None if km is None else bass.AP(
                            tensor=km.tensor,
                            offset=km.offset,
                            ap=[km.ap[0], [0, B], *km.ap[1:]],
                        )
                        rows = slice(1 + alpha, 1 + alpha + QR)
                        eng = nc.gpsimd if idx == pool_idx else nc.vector
                        buf = zpbuf if idx == pool_idx else zbuf
                        tag = "zp" if idx == pool_idx else "z"
                        if bisect == "kconst":
                            # cut the ACT(km)->DVE edge: multiply x by itself
                            kb = None
                        if beta == 0:
                            z = buf.tile([128, B, QR, MCOLS], bf16, tag=tag,
                                         name="z")
                            xin = x_sb[:, cc, :, rows, 0:MCOLS]
                            eng.tensor_mul(z, xin, kb if kb is not None else xin)
                            rhss = [z[:, b, :, 1:65] for b in range(B)]
                        else:
                            c0 = 1 + beta  # 0 or 2, 4B-aligned either way
                            z = buf.tile([128, B, QR, QC], bf16, tag=tag,
                                         name="z")
                            xin = x_sb[:, cc, :, rows, c0 : c0 + QC]
                            eng.tensor_mul(z, xin, kb if kb is not None else xin)
                            rhss = [z[:, b] for b in range(B)]
                        if bisect == "zconst":
                            # cut the DVE(z)->PE edge: matmuls read x directly
                            rhss = [x_sb[:, cc, b, 0:QR, 1:65] for b in range(B)]
                        zs[idx] = rhss
                    if stage <= 1:
                        return
                    # four 8-matmul chains (one per batch) into 1-bank PSUM
                    # tiles; ACT assembles all four into one SBUF tile, one
                    # DMA per class
                    ys = ybuf.tile([F_OUT, 4, 512], bf16, tag="ysb", name="ys")
                    for half in range(2):
                        for b2 in range(2):
                            b = half * 2 + b2
                            yp = ps_y.tile([128, 512], f32, tag="Y", name="yp")
                            for n, i in enumerate(consume_order):
                                nc.tensor.matmul(
                                    yp.rearrange("p (a b) -> p a b", a=QR),
                                    wproj_sb[:, (i // 4) % 2, :],
                                    zs[i][b],
                                    start=(n == 0),
                                    stop=(n == 7),
                                )
                            nc.scalar.activation(
                                ys[:, b], yp, Ident, bias=bb_sb[:, 0:1],
                                scale=1.0,
                            )
                            if stage != 5:
                                # issue per-batch, alternating HWDGE queues,
                                # so transfers spread across the class window
                                eng = nc.sync if b % 2 == 0 else nc.scalar
                                eng.dma_start(y_d[:, ci, b], ys[:, b])

                # deep software pipeline: mlp three classes ahead, kern two
                # classes ahead of the apply stage, so the DVE multiply
                # stream never waits on kern drains and classes overlap.
                n = len(reps)
                hs, ks = {}, {}
                for j in range(min(3, n)):
                    hs[j] = mlp(reps[j]) if stage >= 4 else None
                for j in range(min(2, n)):
                    ks[j] = kerns(reps[j], hs[j])
                for idx, ci in enumerate(reps):
                    if idx + 2 < n:
                        ks[idx + 2] = kerns(reps[idx + 2], hs[idx + 2])
                    if idx + 3 < n:
                        hs[idx + 3] = mlp(reps[idx + 3]) if stage >= 4 else None
                    apply(ci, ks.pop(idx))
                if stage == 1 or stage == 5:
                    ysd = ybuf.tile([F_OUT, 4, 512], bf16, tag="ysb", name="ysd")
                    nc.gpsimd.memset(ysd, 0.0)
                    nc.sync.dma_start(y_d[:, 0], ysd)

            if loop_repeat > 1:
                with tc.For_i(
                    0, loop_repeat, 1,
                    hint_engines=(mybir.EngineType.PE, mybir.EngineType.Activation),
                ):
                    _body_all()
            else:
                _body_all()

    nc.compile()
    return nc


def get_program(repeat: int = 1, loop_repeat: int = 1):
    key = f"nc{repeat}_{loop_repeat}"
    nc = _CACHE.get(key)
    if nc is None:
        nc = _CACHE[key] = _build_program(repeat, loop_repeat)
    return nc


# --------------------------------------------------------------------- entry
def kernel(**inputs) -> np.ndarray:
    nc = get_program()
    in_maps = _host_prep(inputs)
    res = bass_utils.run_bass_kernel_spmd(
        nc, in_maps, core_ids=list(range(N_CORES))
    )
    return _gather(res.results)



# revision 39
# speedup vs baseline: 1.1778x; 1.0308x over previous
"""Trainium2 Bass kernel for nn_CUFLayer_83640193122985.

CUF layer: per-pixel hypernet MLP (118->32->32->32->32->2304) generates 3x3
per-channel kernels at each of 128x128 target pixels; applied to the 2x
nearest-upsampled main_input [4,64,64,256]; then 1x1 projection [256->128].

Key algebraic optimization (parity decomposition): the upsample is exactly 2x
nearest-neighbor, so each output pixel's 3x3 window covers only 2x2 DISTINCT
source pixels; which taps collapse onto which source pixel depends only on the
output pixel's (row, col) parity. W_out/b_out columns are pre-combined on the
host per parity class, turning 9 multiply-taps into 4 and letting the whole
apply stage run at source resolution.

Sharding: 8-way data parallel over output rows (16 rows/core, all batches),
hypernet recomputed per-core for its slab; no collectives. The DCT feature
matrix is input-independent and precomputed on host. Matmuls run in float32r
(near-fp32 precision at full PE rate); the per-pixel multiply runs in bf16 on
the vector engine (2x packed mode, batch-broadcast); tap and channel
accumulation ride the PE's PSUM accumulation fused with the 1x1 projection.
Inputs are packed into few DRAM tensors in need-order (HWDGE dispatch is
~0.65us/DMA, transfers serialize at ~360GB/s); per-class outputs accumulate in
two 2-bank PSUM batch-pair tiles whose copy+DMA overlap each other's matmuls.

Self-contained: hardcodes all shapes; no sibling imports.
"""

import numpy as np
import ml_dtypes

import concourse.bass as bass
import concourse.mybir as mybir
import concourse.tile as tile
from concourse import bacc
from concourse import bass_utils

BF16 = ml_dtypes.bfloat16
F32R = mybir.dt.float32r
F16 = mybir.dt.float16

K = 3
DCT_BASIS = 25
B, H_IN, W_IN, C = 4, 64, 64, 256
H_T, W_T, F_OUT = 128, 128, 128
N_CORES = 8
RPC = H_T // N_CORES  # 16 output rows per core
D_IN = 118
NPIX = RPC * W_T  # 2048 pixels per core
MROWS = RPC // 2 + 2  # 10 source rows incl halo
MCOLS = W_IN + 2  # 66 source cols incl halo
QR = RPC // 2  # 8 source-row positions per core
QC = W_IN  # 64 source-col positions

# vertical tap-collapse table: V[pi][ai] = (alpha, [di...]); same for cols
_V = {0: [(-1, [0]), (0, [1, 2])], 1: [(0, [0, 1]), (1, [2])]}
_CLASSES = [(0, 0), (0, 1), (1, 0), (1, 1)]

_CACHE: dict = {}


# ----------------------------------------------------------------- host side
def _build_features():
    """feat [H_T, W_T, 118] fp32 — input-independent constant."""
    f = np.linspace(1.0, 2.0, DCT_BASIS).astype(np.float32)
    gh = np.linspace(0.0, 1.0, H_T).astype(np.float32)
    row_enc = np.cos(np.pi * (2.0 * gh[:, None] + 1.0) * f[None, :]).astype(np.float32)
    delta = np.concatenate(
        [
            np.broadcast_to(row_enc[:, None, :], (H_T, W_T, DCT_BASIS)),
            np.broadcast_to(row_enc[None, :, :], (H_T, W_T, DCT_BASIS)),
        ],
        axis=-1,
    )
    scale = np.array([H_T / H_IN, W_T / W_IN], np.float32)
    scale_enc = np.cos(np.pi * (2.0 * scale[:, None] + 1.0) * f[None, :]).reshape(-1)
    offs = np.arange(K, dtype=np.float32) - 1.0
    ki, kj = np.meshgrid(offs, offs, indexing="ij")
    kidx = np.stack([ki, kj], -1).reshape(K * K, 2)
    f9 = np.linspace(1.0, 1.0, 9).astype(np.float32)
    kenc = np.cos(np.pi * (2.0 * kidx[..., None] + 1.0) * f9).reshape(K * K, 18).mean(0)
    feat = np.concatenate(
        [
            delta,
            np.broadcast_to(scale_enc, (H_T, W_T, 50)),
            np.broadcast_to(kenc.astype(np.float32), (H_T, W_T, 18)),
        ],
        axis=-1,
    ).astype(np.float32)
    return feat  # [128,128,118]


def _chunk_meta():
    """Per combined-kernel chunk m = class*8 + A*2 + cc: (class, pi, pj,
    alpha, beta, cc, taps). A = ai*2 + bi."""
    meta = []
    for ci, (pi, pj) in enumerate(_CLASSES):
        for ai in range(2):
            for bi in range(2):
                alpha, dis = _V[pi][ai]
                beta, djs = _V[pj][bi]
                taps = [di * 3 + dj for di in dis for dj in djs]
                for cc in range(2):
                    meta.append((ci, pi, pj, alpha, beta, cc, taps))
    return meta


def _host_prep(inputs):
    """Build per-core input maps (few, large tensors to minimize DMA count)."""
    main_input = np.asarray(inputs["main_input"], np.float32)
    feat = _CACHE.get("feat")
    if feat is None:
        feat = _CACHE["feat"] = _build_features()

    # source image, zero-padded by 1: [B, 66, 66, C] then channel-major bf16
    mp = np.pad(main_input, ((0, 0), (1, 1), (1, 1), (0, 0)))

    Wout = np.asarray(inputs["W_out"], np.float32)  # [32, 2304] cols t*256+c
    bout = np.asarray(inputs["b_out"], np.float32)
    wcomb = np.empty((32, 32 * 128), np.float32)
    bcomb = np.empty((128, 32), np.float32)
    for m, (ci, pi, pj, al, be, cc, taps) in enumerate(_chunk_meta()):
        Wc = sum(Wout[:, t * 256 + cc * 128 : t * 256 + (cc + 1) * 128] for t in taps)
        bc = sum(bout[t * 256 + cc * 128 : t * 256 + (cc + 1) * 128] for t in taps)
        wcomb[:, m * 128 : (m + 1) * 128] = Wc
        bcomb[:, m] = bc

    # wm: w2 | w3 | w4ext | wcomb_ext  -> [33, 97 + 4096]; row 32 carries bcomb
    # (the kern matmul consumes an appended ones-row in h4, folding the bias
    # into the PE accumulation at zero cost)
    wm = np.zeros((33, 97 + 32 * 128), np.float32)
    wm[:32, 0:32] = np.asarray(inputs["W2"], np.float32)
    wm[:32, 32:64] = np.asarray(inputs["W3"], np.float32)
    # W4 gets a 33rd output column of zeros; with bias 1.0 it yields the
    # constant ones-row in h4 that carries bcomb through the kern matmul
    wm[:32, 64:96] = np.asarray(inputs["W4"], np.float32)
    wm[:32, 97:] = wcomb
    wm[32, 97:] = bcomb.T.reshape(-1)
    # bs: b1..b4 -> [33, 4]; bs[32, 3] = 1.0 feeds the h4 ones-row
    bs = np.zeros((33, 4), np.float32)
    for i in (1, 2, 3, 4):
        bs[:32, i - 1] = np.asarray(inputs[f"b{i}"], np.float32)
    bs[32, 3] = 1.0
    bb = np.asarray(inputs["b_proj"], np.float32).reshape(128, 1)
    wproj = np.ascontiguousarray(
        np.asarray(inputs["W_proj"], np.float32).reshape(2, 128, F_OUT).transpose(1, 0, 2)
    ).astype(BF16)  # [128c, 2cc, F]

    w1 = np.asarray(inputs["W1"], np.float32)  # [118, 32]
    in_maps = []
    for k in range(N_CORES):
        m0 = k * QR  # first source row of this core's slab
        slab = mp[:, m0 : m0 + MROWS, :, :]  # [B,10,66,C]
        x_cm = np.ascontiguousarray(slab.transpose(3, 0, 1, 2)).reshape(
            2, 128, B, MROWS, MCOLS
        ).astype(BF16)
        # feature columns grouped by parity class: (class, q, j); append W1
        r0 = k * RPC
        fs = feat[r0 : r0 + RPC]  # [16,128,118]
        fcls = np.stack(
            [fs[pi::2, pj::2].reshape(QR * QC, D_IN) for (pi, pj) in _CLASSES]
        )  # [4, 512, 118]
        fw1 = np.concatenate(
            [w1, np.ascontiguousarray(fcls.reshape(4 * QR * QC, D_IN).T)], axis=1
        ).astype(np.float16)  # [118, 2080] = [w1 | feat]
        in_maps.append({"x": x_cm, "fw1": fw1, "wm": wm.astype(np.float16),
                        "bs": bs, "bb": bb, "wproj": wproj})
    return in_maps


def _gather(results):
    """results[k]["y"] [F, 4class, B, 512] -> [B, H_T, W_T, F] fp32."""
    out = np.empty((B, H_T, W_T, F_OUT), np.float32)
    for k, res in enumerate(results):
        y5 = res["y"].reshape(F_OUT, 4, B, QR, QC)
        slab = out[:, k * RPC : (k + 1) * RPC]  # [B,16,128,F] view
        for ci, (pi, pj) in enumerate(_CLASSES):
            slab[:, pi::2, pj::2] = y5[:, ci].transpose(1, 2, 3, 0)
    return out


# -------------------------------------------------------------- device program
def _build_program(repeat: int = 1, loop_repeat: int = 1):
    f32, bf16 = mybir.dt.float32, mybir.dt.bfloat16
    Relu = mybir.ActivationFunctionType.Relu
    Ident = mybir.ActivationFunctionType.Identity

    nc = bacc.Bacc("TRN2", target_bir_lowering=False, debug=False, num_devices=N_CORES)
    x_d = nc.dram_tensor("x", (2, 128, B, MROWS, MCOLS), bf16, kind="ExternalInput")
    fw1_d = nc.dram_tensor("fw1", (D_IN, NPIX + 32), F16, kind="ExternalInput")
    wm_d = nc.dram_tensor("wm", (33, 97 + 32 * 128), F16, kind="ExternalInput")
    bs_d = nc.dram_tensor("bs", (33, 4), f32, kind="ExternalInput")
    bb_d = nc.dram_tensor("bb", (128, 1), f32, kind="ExternalInput")
    wproj_d = nc.dram_tensor("wproj", (128, 2, F_OUT), bf16, kind="ExternalInput")
    y_d = nc.dram_tensor("y", (F_OUT, 4, B, 512), bf16, kind="ExternalOutput")

    meta = _chunk_meta()

    with tile.TileContext(nc) as tc:
        with (
            tc.tile_pool(name="const", bufs=1) as const,
            tc.tile_pool(name="hbuf", bufs=2) as hbuf,
            tc.tile_pool(name="kern", bufs=3) as kern_pool,
            tc.tile_pool(name="zbuf", bufs=16) as zbuf,
            tc.tile_pool(name="ybuf", bufs=2) as ybuf,
            # 8 PSUM banks: mlp 2x1 + kern 2x2 + y 2x1. Decoupled pools keep
            # the mlp chain, kern production, and apply output from
            # serializing each other through buffer rotation.
            tc.tile_pool(name="ps_mlp", bufs=2, space="PSUM") as ps_mlp,
            tc.tile_pool(name="ps_kern", bufs=2, space="PSUM") as ps_kern,
            tc.tile_pool(name="ps_y", bufs=2, space="PSUM") as ps_y,
        ):
            # ---- input loads, in need-order: MLP biases + w1 + class-0
            # features, MLP/kern weights, the image, remaining features,
            # projection weights ----
            bs_sb = const.tile([33, 4], f32)
            nc.sync.dma_start(bs_sb, bs_d[:])
            fw1_sb = const.tile([D_IN, NPIX + 32], F16)
            nc.sync.dma_start(fw1_sb[:, 0:544], fw1_d[:, 0:544])
            wm_sb = const.tile([33, 97 + 32 * 128], F16)
            nc.sync.dma_start(wm_sb, wm_d[:])
            x_sb = const.tile([128, 2, B, MROWS, MCOLS], bf16)
            nc.sync.dma_start(x_sb, x_d[:].transpose((1, 0, 2, 3, 4)))
            nc.sync.dma_start(fw1_sb[:, 544:], fw1_d[:, 544:])
            wproj_sb = const.tile([128, 2, F_OUT], bf16)
            nc.sync.dma_start(wproj_sb, wproj_d[:])
            bb_sb = const.tile([128, 1], f32)
            nc.sync.dma_start(bb_sb, bb_d[:])

            w_sb = {
                1: fw1_sb[:, 0:32],
                2: wm_sb[0:32, 0:32],
                3: wm_sb[0:32, 32:64],
                4: wm_sb[0:32, 64:97],
            }
            wcomb_sb = wm_sb[:, 97 : 97 + 32 * 128]

            def _body_all():
                import os
                stage = int(os.environ.get("K_STAGE", "4"))
                # 1: DVE multiplies only (const k); 2: +apply/ys/DMA;
                # 3: +kern matmuls/drains (const h); 4: full; 5: full, no DMA
                reps = [c for _ in range(repeat) for c in range(4)]

                def mlp_layer(ci, i, h):
                    rows = 33 if i == 4 else 32
                    ps = ps_mlp.tile([rows, 512], f32, tag="mlp", name="ps")
                    nc.tensor.matmul(ps, w_sb[i], h, start=True, stop=True)
                    hn = hbuf.tile([rows, 512], F16, tag=f"h{ci}", name="hn")
                    nc.scalar.activation(
                        hn, ps, Relu, bias=bs_sb[0:rows, i - 1 : i], scale=1.0
                    )
                    return hn

                def mlp(ci):
                    h = fw1_sb[:, 32 + ci * 512 : 32 + (ci + 1) * 512]
                    for i in range(1, 5):
                        h = mlp_layer(ci, i, h)
                    return h

                def kern_piece(ci, A, h, tiles):
                    if os.environ.get("K_BISECT", "") == "hconst" or stage == 3:
                        h = wm_sb[0:33, 0:512]  # cut the MLP->kern edge
                    m0 = ci * 8 + A * 2
                    _, _, _, alpha, beta, _, _ = meta[m0]
                    ps = ps_kern.tile([128, 1024], f32, tag="kps", name="ps")
                    for cc in range(2):
                        nc.tensor.matmul(
                            ps[:, cc * 512 : (cc + 1) * 512],
                            wcomb_sb[:, (m0 + cc) * 128 : (m0 + cc + 1) * 128],
                            h,
                            start=True,
                            stop=True,
                        )
                    ps28 = ps.rearrange("p (c a b) -> p c a b", c=2, a=QR)
                    wide = MCOLS if beta == 0 else QC
                    km = kern_pool.tile(
                        [128, 2, QR, wide], bf16, tag=f"k{A}", name="km"
                    )
                    if beta == 0:
                        border = bass.AP(
                            tensor=km.tensor,
                            offset=km.offset,
                            ap=[km.ap[0], km.ap[1], km.ap[2], [65, 2]],
                        )
                        nc.gpsimd.memset(border, 0.0)
                        nc.scalar.copy(km[:, :, :, 1:65], ps28)
                    else:
                        nc.scalar.copy(km, ps28)
                    for cc in range(2):
                        tiles[A * 2 + cc] = (km[:, cc], alpha, beta)

                def kerns(ci, h):
                    if stage <= 2:
                        return {
                            A * 2 + cc: (None, meta[ci * 8 + A * 2][3],
                                         meta[ci * 8 + A * 2][4])
                            for A in range(4) for cc in range(2)
                        }
                    tiles = {}
                    for A in range(4):
                        kern_piece(ci, A, h, tiles)
                    return tiles

                # A-major order: each km drain (per A) immediately enables
                # the next two multiplies, minimizing drain->DVE stalls
                consume_order = [0, 4, 1, 5, 2, 6, 3, 7]
                bisect = os.environ.get("K_BISECT", "")

                def emit_z(ci, kern_tiles):
                    zs = [None] * 8
                    for idx in consume_order:
                        cc, A = idx // 4, idx % 4
                        km, alpha, beta = kern_tiles[A * 2 + cc]
                        kb = None if km is None else bass.AP(
                            tensor=km.tensor,
                            offset=km.offset,
                            ap=[km.ap[0], [0, B], *km.ap[1:]],
                        )
                        rows = slice(1 + alpha, 1 + alpha + QR)
                        if bisect == "kconst":
                            # cut the ACT(km)->DVE edge: multiply x by itself
                            kb = None
                        if beta == 0:
                            z = zbuf.tile([128, B, QR, MCOLS], bf16, tag="z",
                                          name="z")
                            xin = x_sb[:, cc, :, rows, 0:MCOLS]
                            nc.vector.tensor_mul(
                                z, xin, kb if kb is not None else xin)
                            rhss = [z[:, b, :, 1:65] for b in range(B)]
                        else:
                            c0 = 1 + beta  # 0 or 2, 4B-aligned either way
                            z = zbuf.tile([128, B, QR, QC], bf16, tag="z",
                                          name="z")
                            xin = x_sb[:, cc, :, rows, c0 : c0 + QC]
                            nc.vector.tensor_mul(
                                z, xin, kb if kb is not None else xin)
                            rhss = [z[:, b] for b in range(B)]
                        if bisect == "zconst":
                            # cut the DVE(z)->PE edge: matmuls read x directly
                            rhss = [x_sb[:, cc, b, 0:QR, 1:65] for b in range(B)]
                        zs[idx] = rhss
                    return zs

                def apply_chain(ci, b, zs, ys):
                    # one 8-matmul accumulation chain (one batch) into a
                    # 1-bank PSUM tile, drained into the class's ys tile
                    yp = ps_y.tile([128, 512], f32, tag="Y", name="yp")
                    for nn, i in enumerate(consume_order):
                        nc.tensor.matmul(
                            yp.rearrange("p (a b) -> p a b", a=QR),
                            wproj_sb[:, (i // 4) % 2, :],
                            zs[i][b],
                            start=(nn == 0),
                            stop=(nn == 7),
                        )
                    nc.scalar.activation(
                        ys[:, b], yp, Ident, bias=bb_sb[:, 0:1], scale=1.0
                    )
                    if stage != 5:
                        # alternate HWDGE queues; transfers spread across
                        # the class window
                        eng = nc.sync if b % 2 == 0 else nc.scalar
                        eng.dma_start(y_d[:, ci, b], ys[:, b])

                # Software pipeline with interleaved PE emission: during
                # class c's four apply chains, the kern pieces for class c+2
                # and the mlp layers for class c+3 are woven between chains,
                # so PE's in-order stream never sits idle on a kern psum
                # rotation or an mlp ACT round-trip, and the DVE multiply
                # stream (one full class ahead) is never blocked on drains.
                n = len(reps)
                hs, ks = {}, {}
                for j in range(min(2, n)):
                    hs[j] = mlp(reps[j]) if stage >= 4 else None
                    ks[j] = kerns(reps[j], hs[j])
                if n > 2 and stage >= 4:
                    hs[2] = mlp(reps[2])
                for idx, ci in enumerate(reps):
                    zs = emit_z(ci, ks.pop(idx))
                    if stage <= 1:
                        continue
                    weave_k = idx + 2 < n and stage >= 3
                    weave_m = idx + 3 < n and stage >= 4
                    if weave_k:
                        ks[idx + 2] = {}
                    elif idx + 2 < n:
                        ks[idx + 2] = kerns(reps[idx + 2], None)
                    if weave_m:
                        mh = fw1_sb[:, 32 + reps[idx + 3] * 512
                                    : 32 + (reps[idx + 3] + 1) * 512]
                    ys = ybuf.tile([F_OUT, 4, 512], bf16, tag="ysb", name="ys")
                    for b in range(4):
                        if weave_k:
                            kern_piece(reps[idx + 2], b, hs.get(idx + 2),
                                       ks[idx + 2])
                        if weave_m:
                            mh = mlp_layer(reps[idx + 3], b + 1, mh)
                        apply_chain(ci, b, zs, ys)
                    if weave_m:
                        hs[idx + 3] = mh
                if stage == 1 or stage == 5:
                    ysd = ybuf.tile([F_OUT, 4, 512], bf16, tag="ysb", name="ysd")
                    nc.gpsimd.memset(ysd, 0.0)
                    nc.sync.dma_start(y_d[:, 0], ysd)

            if loop_repeat > 1:
                with tc.For_i(
                    0, loop_repeat, 1,
                    hint_engines=(mybir.EngineType.PE, mybir.EngineType.Activation),
                ):
                    _body_all()
            else:
                _body_all()

    nc.compile()
    return nc


def get_program(repeat: int = 1, loop_repeat: int = 1):
    key = f"nc{repeat}_{loop_repeat}"
    nc = _CACHE.get(key)
    if nc is None:
        nc = _CACHE[key] = _build_program(repeat, loop_repeat)
    return nc


# --------------------------------------------------------------------- entry
def kernel(**inputs) -> np.ndarray:
    nc = get_program()
    in_maps = _host_prep(inputs)
    res = bass_utils.run_bass_kernel_spmd(
        nc, in_maps, core_ids=list(range(N_CORES))
    )
    return _gather(res.results)

